# revision 29
# baseline (speedup 1.0000x reference)
"""Causal self-attention (B=8, T=1024, C=768, NH=12) on 8 TRN2 NeuronCores.

Strategy: pure batch data-parallel - core b computes batch element b end to
end (no collectives).

v4 (baseline 266909 -> v2 248375 -> v3 165219 -> this):
  * all weights preloaded up front (12 wqk chunks + wv + wp) on the sync
    queue - no mid-phase weight DMA waits.
  * startup: x chunk 0 DMA'd first, qkt chunks 0/6 accumulate in x-arrival
    order (cc-pair interleaved across 4 psum groups), few small dummies
    warm the PE clock.
  * causal masks (gpsimd affine_select) are the ONLY gpsimd work;
    the softmax-denominator DMA hops moved to the sync queue (idle in the
    attention phase) so masks never queue behind them - this was the v3
    engine stall that tripped HAM down to half clock.
  * one spread matmul per block: recips land as a [2, TQ] fp16 tile
    (par0 row 0, par1 row 1), spread to [128, TQ] by a K=2 selector
    matmul; rawT even/odd halves are plain DVE muls (mixed partition
    offsets are legal) - v3's odd-half DMA reshape hop is gone.
  * last two blocks skip the DMA hops: direct 1-partition DVE recips
    (partition 64 -> 0) halve the tail chain latency.
  * block order: all (g, t2=0) halves first, then all (g, t2=1); proj
    t2=0 chunks become PE filler work late in the t2=1 phase, and two
    reserved proj chunks cover the final denominator chains.
  * evac engine split: qkt phase-A evacs + per-block par0 evac on ACT,
    par1 evac + norm muls + filler qkt evacs on DVE.
  * yT output in bf16 (host upcasts) - halves the output drain.

Per-core dataflow (everything kept "transposed", i.e. [feature, time]):
  xT [C, T] bf16                             (host pre-transposes x[b])
  qkT[j, t] = Wqk[:, j].T x                  attT-friendly layout
  v  [t, j] = x Wv                           AV-friendly layout, one ones
                                             column per head (denominator)
  attT[tk, tq] = kT.T @ qT   per head pair   PSUM [128, 2, 512]
  expT = exp(scale * attT)                   no max-sub: |logits| small
  diag blocks: affine_select (gpsimd)        causal mask
  out_aug[d|denom] = [v | 1].T @ expT        M=65, psum row 64 = denom
  rawT[j, t] = out_aug[d] * (1/denom)        K=2 selector spread matmul
  yT[e, t] = Wp.T @ rawT + bp                bf16 out, host transposes
"""

import os
import sys
from collections import deque

import numpy as np

for _p in ("/opt/trn_rl_repo", "/root/.axon_site/_ro/trn_rl_repo"):
    if os.path.isdir(_p) and _p not in sys.path:
        sys.path.insert(0, _p)

import ml_dtypes

import concourse.bacc as bacc
import concourse.mybir as mybir
import concourse.tile as tile
from concourse.bass import ts
from concourse.bass_utils import run_bass_kernel_spmd

B, T, C = 8, 1024, 768
NH, HD = 12, 64
P = 128
NCORES = 8
CC = 6                 # contraction chunks over C
JQK = 12               # output chunks for q|k
EC = 6                 # output chunks for the projection
TQ = 512               # moving-dim tile (max psum bank width)
NTQ = 2
NTK = 8                # key chunks
G = 6                  # head pairs (two 64-wide heads per 128 partitions)
VW = 2 * HD + 2        # 130: per-pair v layout [d_even(64), 1, d_odd(64), 1]
JV = 384               # v output tile width (3 head pairs)
SCALE = 1.0 / float(np.sqrt(HD))
NDUM = 6               # warmup dummy matmuls
DUMN = 512
POPS = 3               # filler pops per AV pair
F32 = mybir.dt.float32
BF16 = mybir.dt.bfloat16
F16 = mybir.dt.float16
AF = mybir.ActivationFunctionType
ADD = mybir.AluOpType.add
MULT = mybir.AluOpType.mult

_CACHE = {}


def _build():
    if "nc" in _CACHE:
        return _CACHE["nc"]

    nc = bacc.Bacc("TRN2", target_bir_lowering=False, debug=False)

    xT = nc.dram_tensor("xT", [C, T], BF16, kind="ExternalInput")
    wqk = nc.dram_tensor("wqk", [C, 2 * C], BF16, kind="ExternalInput")
    wv = nc.dram_tensor("wv", [C, C], BF16, kind="ExternalInput")
    wp = nc.dram_tensor("wp", [C, C], BF16, kind="ExternalInput")
    cst = nc.dram_tensor("cst", [P, 18], F32, kind="ExternalInput")
    cstb = nc.dram_tensor("cstb", [P, 1024], BF16, kind="ExternalInput")
    csth = nc.dram_tensor("csth", [2, 384], F16, kind="ExternalInput")
    yT = nc.dram_tensor("yT", [C, T], BF16, kind="ExternalOutput")

    xT_r = xT[:].rearrange("(h p) t -> p h t", p=P)  # h = 6 chunks
    wqk_r = wqk[:].rearrange("(o p) j -> p o j", p=P)
    wv_r = wv[:].rearrange("(o p) j -> p o j", p=P)
    wp_r = wp[:].rearrange("(o p) e -> p o e", p=P)
    yT_r = yT[:].rearrange("(o p) t -> p o t", p=P)

    with tile.TileContext(nc) as tc:
        with (
            tc.tile_pool(name="const", bufs=1) as constp,
            tc.tile_pool(name="xt", bufs=3) as xtp,
            tc.tile_pool(name="wqk", bufs=12) as wqkp,
            tc.tile_pool(name="wv", bufs=1) as wvp,
            tc.tile_pool(name="wp", bufs=1) as wpp,
            tc.tile_pool(name="qkt", bufs=1) as qkTp,
            tc.tile_pool(name="vaug", bufs=1) as vap,
            tc.tile_pool(name="raw", bufs=1) as rawp,
            tc.tile_pool(name="exp", bufs=6) as expp,
            tc.tile_pool(name="asb", bufs=3) as asbp,
            tc.tile_pool(name="rr", bufs=4) as rrp,
            tc.tile_pool(name="yt", bufs=3) as ytp,
            tc.tile_pool(name="psA", bufs=2, space="PSUM") as psA,
            tc.tile_pool(name="psB", bufs=4, space="PSUM") as psB,
        ):
            # ---- warmup: keep the PE clock gate open until real work ----
            dumw = constp.tile([1, DUMN], BF16)
            nc.gpsimd.memset(dumw[:], 0.0)
            dups = psB.tile([1, DUMN], F32, tag="mm", name="dups")
            for _ in range(NDUM):
                nc.tensor.matmul(
                    dups[0:1, :], dumw[0:1, 0:1], dumw[0:1, :],
                    start=True, stop=True,
                )

            # ---- all input DMAs, spread across 4 engine DMA rings -------
            # a single ring sustains well under the per-core HBM rate; the
            # startup-critical x chunks + first weights go wide in parallel
            xt2s = [
                xtp.tile([P, 2, T], BF16, tag="xt", name=f"xt{h}")
                for h in range(3)
            ]
            xts = [xt2s[cc // 2][:, cc % 2, :] for cc in range(CC)]

            wt = [
                wqkp.tile([P, CC, P], BF16, tag="wqk", name=f"wt{j}")
                for j in range(JQK)
            ]

            def load_wt(jc, eng=nc.sync):
                eng.dma_start(wt[jc][:], wqk_r[:, :, ts(jc, P)])

            load_wt(0, nc.scalar)
            nc.sync.dma_start(xt2s[0][:], xT_r[:, 0:2, :])
            load_wt(G, nc.scalar)
            nc.sync.dma_start(xt2s[1][:], xT_r[:, 2:4, :])
            nc.sync.dma_start(xt2s[2][:], xT_r[:, 4:6, :])

            wv_sb = wvp.tile([P, CC, C], BF16)
            nc.sync.dma_start(wv_sb[:, 0:3, :], wv_r[:, 0:3, :])
            nc.sync.dma_start(wv_sb[:, 3:6, :], wv_r[:, 3:6, :])
            cst_sb = constp.tile([P, 18], F32)
            nc.scalar.dma_start(cst_sb[:], cst[:])
            load_wt(1, nc.scalar)
            load_wt(G + 1, nc.scalar)
            # prime the ACT activation table during the DMA-paced startup
            # (the lazy ACT_TABLE_LOAD costs 1.3us on the first ACT op)
            dumact = constp.tile([1, 1], F32)
            nc.scalar.copy(dumact[:], dumw[0:1, 0:1])

            cstb_sb = constp.tile([P, 1024], BF16)
            nc.sync.dma_start(cstb_sb[:], cstb[:])
            csth_sb = constp.tile([2, 384], F16)
            nc.sync.dma_start(csth_sb[:], csth[:])
            bqk_sb = cst_sb[:, 0:JQK]
            bp_sb = cst_sb[:, JQK : JQK + EC]
            bv_sb = cstb_sb[:, 0:C]
            sel2 = csth_sb[0:2, 0:128]
            selA = csth_sb[0:1, 128:256]
            selB = csth_sb[0:1, 256:384]

            wp_sb = wpp.tile([P, CC, C], BF16)
            nc.sync.dma_start(wp_sb[:, 0:3, :], wp_r[:, 0:3, :])
            nc.sync.dma_start(wp_sb[:, 3:6, :], wp_r[:, 3:6, :])
            for j in (2, 8, 3, 9, 4, 10, 5, 11):
                load_wt(j, nc.sync)

            qkT_sb = qkTp.tile([P, JQK, T], BF16)
            v_sb = vap.tile([P, NTK, G * VW], BF16)
            v4 = v_sb[:].rearrange("p n (g w) -> p n g w", w=VW)
            rawT = rawp.tile([P, CC, T], BF16)

            # ones columns feed the softmax-denominator trick
            onec = constp.tile([P, 1], F32)
            nc.vector.memset(onec[:], 1.0)
            ones_src = onec[:, None, None, :].to_broadcast([P, NTK, G, 1])
            nc.vector.tensor_copy(v4[:, :, :, HD : HD + 1], ones_src)
            nc.vector.tensor_copy(v4[:, :, :, VW - 1 : VW], ones_src)

            # ---- qkt chunks 0/6 in x-arrival order ----------------------
            psq4 = {}
            for j in (0, G):
                for t2 in range(NTQ):
                    psq4[(j, t2)] = psB.tile(
                        [P, TQ], F32, tag="mm", name="psq"
                    )
            # emission order matches DMA arrival (wt0/wt6 land first)
            for j, p3 in ((0, 0), (G, 0), (0, 1), (G, 1), (0, 2), (G, 2)):
                for t2 in range(NTQ):
                    for cc in (2 * p3, 2 * p3 + 1):
                        nc.tensor.matmul(
                            psq4[(j, t2)][:],
                            wt[j][:, cc, :],
                            xts[cc][:, ts(t2, TQ)],
                            start=(cc == 0),
                            stop=(cc == CC - 1),
                        )
            for j in (0, G):
                for t2 in range(NTQ):
                    nc.scalar.add(
                        qkT_sb[:, j, ts(t2, TQ)],
                        psq4[(j, t2)][:],
                        bqk_sb[:, j : j + 1],
                    )

            # ---- v phase -----------------------------------------------
            for tc_i in range(NTK):
                for jn in range(C // JV):
                    ps = psB.tile([P, TQ], F32, tag="mm", name="psv")
                    for cc in range(CC):
                        nc.tensor.matmul(
                            ps[:, :JV],
                            xts[cc][:, ts(tc_i, P)],
                            wv_sb[:, cc, ts(jn, JV)],
                            start=(cc == 0),
                            stop=(cc == CC - 1),
                        )
                    g0 = jn * (JV // P)  # 3 head pairs per 384 cols
                    srcv = ps[:, :JV].rearrange(
                        "p (g h d) -> p g h d", h=2, d=HD
                    )
                    bias = bv_sb[:, ts(jn, JV)].rearrange(
                        "p (g h d) -> p g h d", h=2, d=HD
                    )
                    nc.vector.tensor_tensor(
                        v4[:, tc_i, g0 : g0 + 3, 0:HD],
                        srcv[:, :, 0, :],
                        bias[:, :, 0, :],
                        ADD,
                    )
                    nc.vector.tensor_tensor(
                        v4[:, tc_i, g0 : g0 + 3, HD + 1 : VW - 1],
                        srcv[:, :, 1, :],
                        bias[:, :, 1, :],
                        ADD,
                    )

            # ---- qkt chunks 1/7 ----------------------------------------
            def qkt_chunk(jc):
                for t2 in range(NTQ):
                    ps = psB.tile([P, TQ], F32, tag="mm", name="psq")
                    for cc in range(CC):
                        nc.tensor.matmul(
                            ps[:],
                            wt[jc][:, cc, :],
                            xts[cc][:, ts(t2, TQ)],
                            start=(cc == 0),
                            stop=(cc == CC - 1),
                        )
                    nc.scalar.add(
                        qkT_sb[:, jc, ts(t2, TQ)],
                        ps[:],
                        bqk_sb[:, jc : jc + 1],
                    )

            # ---- fillers: remaining qkt chunks + (later) proj t2=0 ------
            fillq = []

            remaining = {}

            def qkt_fillers(jc, t2):
                state = {}
                key = (jc, t2)
                remaining[key] = CC

                def mk(cc, state=state):
                    def run():
                        if cc == 0:
                            state["ps"] = psB.tile(
                                [P, TQ], F32, tag="mm", name="psqf"
                            )
                        ps = state["ps"]
                        nc.tensor.matmul(
                            ps[:],
                            wt[jc][:, cc, :],
                            xts[cc][:, ts(t2, TQ)],
                            start=(cc == 0),
                            stop=(cc == CC - 1),
                        )
                        if cc == CC - 1:
                            nc.vector.tensor_scalar_add(
                                qkT_sb[:, jc, ts(t2, TQ)],
                                ps[:],
                                bqk_sb[:, jc : jc + 1],
                            )

                    return run

                fillq.extend((key, mk(cc)) for cc in range(CC))

            for t2 in range(NTQ):  # all t2=0 halves first
                for g2 in (1, 2, 3, 4, 5):
                    qkt_fillers(g2, t2)
                    qkt_fillers(G + g2, t2)

            def pop_fill(k):
                for _ in range(k):
                    if fillq:
                        key, fn = fillq.pop(0)
                        if key is not None:
                            remaining[key] -= 1
                        fn()

            def drain_for(keys):
                # emit every filler a block depends on before its first QK
                while fillq and any(remaining.get(k, 0) > 0 for k in keys):
                    pop_fill(1)

            fill0 = nc.gpsimd.to_reg(0.0)

            def finish_head(state):
                """Denominator chain, no PE work: [P, 8] transpose hop on
                the sync queue, parallel fp16 recip, hop back to [2, TQ]."""
                g, t2, asb = state
                rd = rrp.tile([P, 8], F32, tag="rd", name="rd")
                nc.sync.dma_start(rd[:], asb[64:65, :, :])
                rd2 = rrp.tile([P, 8], F16, tag="rd2", name="rd2")
                with nc.allow_low_precision(
                    reason="fp16 softmax denominators keep 11 bits"
                ):
                    nc.vector.reciprocal(rd2[:], rd[:])
                rro2 = rrp.tile([2, TQ], F16, tag="rro2", name="rro2")
                nc.sync.dma_start(rro2[:], rd2[:])
                return g, t2, asb, rro2

            def finish_tail(h):
                """K=2 selector spread matmul + the two normalize muls."""
                g, t2, asb, rro2 = h
                prs = psB.tile([P, TQ], F32, tag="mm", name="prs")
                nc.tensor.matmul(prs[:], sel2, rro2[:], start=True, stop=True)
                nc.vector.tensor_mul(
                    rawT[0:64, g, ts(t2, TQ)], asb[0:64, 0, :], prs[0:64, :]
                )
                nc.vector.tensor_mul(
                    rawT[64:128, g, ts(t2, TQ)], asb[0:64, 1, :],
                    prs[64:128, :],
                )

            def attn_finish(state):
                finish_tail(finish_head(state))

            def proj_piece(t2, ec, eng, ps=None):
                if ps is None:
                    ps = psB.tile([P, TQ], F32, tag="mm", name="psp")
                for jc in range(CC):
                    nc.tensor.matmul(
                        ps[:],
                        wp_sb[:, jc, ts(ec, P)],
                        rawT[:, jc, ts(t2, TQ)],
                        start=(jc == 0),
                        stop=(jc == CC - 1),
                    )
                yt = ytp.tile([P, TQ], BF16, tag="yt", name="yt")
                if eng == "act":
                    nc.scalar.add(yt[:], ps[:], bp_sb[:, ec : ec + 1])
                else:
                    nc.vector.tensor_scalar_add(
                        yt[:], ps[:], bp_sb[:, ec : ec + 1]
                    )
                nc.sync.dma_start(yT_r[:, ec, ts(t2, TQ)], yt[:])

            def proj_fillers(t2, ec):
                state = {}

                def mk(jc, state=state):
                    def run():
                        if jc == 0:
                            state["ps"] = psB.tile(
                                [P, TQ], F32, tag="mm", name="pspf"
                            )
                        ps = state["ps"]
                        nc.tensor.matmul(
                            ps[:],
                            wp_sb[:, jc, ts(ec, P)],
                            rawT[:, jc, ts(t2, TQ)],
                            start=(jc == 0),
                            stop=(jc == CC - 1),
                        )
                        if jc == CC - 1:
                            yt = ytp.tile([P, TQ], BF16, tag="yt", name="yt")
                            nc.vector.tensor_scalar_add(
                                yt[:], ps[:], bp_sb[:, ec : ec + 1]
                            )
                            nc.sync.dma_start(
                                yT_r[:, ec, ts(t2, TQ)], yt[:]
                            )

                    return run

                fillq.extend((None, mk(jc)) for jc in range(CC))

            # ---- attention: flat pipeline across all 12 blocks ----------
            # The last AV pair of block n is emitted after block n+1's
            # first QK, so the PE never faces a block-start exp+mask chain
            # with nothing in front of it. Filler pops go BEFORE each
            # carried AV (in-order queue: work behind a waiting AV is
            # stuck, work in front is not).
            sched = [(g, 0) for g in range(G)] + [(g, 1) for g in range(G)]
            carryq = deque()   # (g, t2, hi, e, cs, tkc), AV lags 2 tkcs
            avs_cur = [None, None]
            heads = []         # denominator chains in flight
            ntail = 0

            def do_av(c):
                g, t2, hi, e, cs, tkc = c
                if tkc == 0:
                    avs_cur[0] = psB.tile([P, TQ], F32, tag="mm", name="av0")
                    avs_cur[1] = psB.tile([P, TQ], F32, tag="mm", name="av1")
                for par in (0, 1):
                    vlo = g * VW + (HD + 1) * par
                    nc.tensor.matmul(
                        avs_cur[par][0:65, cs:],
                        v_sb[:, tkc, vlo : vlo + HD + 1],
                        e[:, par, cs:],
                        start=(tkc == 0),
                        stop=(tkc == hi - 1),
                    )
                if tkc == hi - 1:
                    asb = asbp.tile(
                        [65, 2, TQ], F32, tag="avsb", name="asb"
                    )
                    nc.vector.tensor_scalar_add(
                        asb[:, 0, :], avs_cur[0][0:65, :], 0.0
                    )
                    nc.vector.tensor_scalar_add(
                        asb[:, 1, :], avs_cur[1][0:65, :], 0.0
                    )
                    # start the denominator chain immediately (no PE work)
                    heads.append(finish_head((g, t2, asb)))

            for bi, (g, t2) in enumerate(sched):
                jq, jk = g, G + g
                hi = 4 * (t2 + 1)  # causal: key chunks 0..hi-1
                need = [(g, t2), (G + g, t2)]
                if t2 == 1:
                    need.append((G + g, 0))
                drain_for(need)
                for tkc in range(hi):
                    csr = tkc * P - t2 * TQ  # diag block start col
                    cs = max(0, csr)
                    pa = psA.tile([P, 2, TQ], F32, tag="pa", name="pa")
                    for par in (0, 1):
                        qrow = HD * par
                        nc.tensor.matmul(
                            pa[:, par, cs:TQ],
                            qkT_sb[qrow : qrow + HD, jk, ts(tkc, P)],
                            qkT_sb[
                                qrow : qrow + HD,
                                jq,
                                t2 * TQ + cs : (t2 + 1) * TQ,
                            ],
                            start=True,
                            stop=True,
                        )
                    pop_fill(2 if tkc < 2 else 1)
                    e = expp.tile([P, 2, TQ], BF16, tag="exp", name="e")
                    nc.scalar.activation(
                        e[:, :, cs:], pa[:, :, cs:], AF.Exp, scale=SCALE
                    )
                    if csr >= 0:
                        # causal mask on the diagonal 128-wide block: keep
                        # e[tk, tq] only where tq_local >= tk_local
                        nc.gpsimd.affine_select(
                            e[:, :, cs : cs + P],
                            e[:, :, cs : cs + P],
                            pattern=[[0, 2], [1, P]],
                            compare_op=mybir.AluOpType.is_ge,
                            fill=fill0,
                            base=0,
                            channel_multiplier=-1,
                        )
                    if len(carryq) == 4:
                        do_av(carryq.popleft())
                        if tkc < 3:
                            pop_fill(1)
                    carryq.append((g, t2, hi, e, cs, tkc))
                    if len(heads) >= 2 and ntail < 10:
                        finish_tail(heads.pop(0))
                        ntail += 1
                if bi == 9:
                    for ec in range(2):
                        proj_fillers(0, ec)
            while carryq:
                do_av(carryq.popleft())
            # tail: both remaining chains are already in flight; keep the
            # PE busy on reserved proj chunks while they run. Those chunks
            # accumulate in retired pa (psA) banks so they never wait on
            # the psB ring still entangled with the last block's AV banks.
            pop_fill(999)
            pp1 = psA.tile([P, 2, TQ], F32, tag="pa", name="pp1")
            pp2 = psA.tile([P, 2, TQ], F32, tag="pa", name="pp2")
            proj_piece(0, 2, "act", ps=pp1[:, 0, :])
            proj_piece(0, 3, "act", ps=pp1[:, 1, :])
            finish_tail(heads.pop(0))
            proj_piece(0, 4, "act", ps=pp2[:, 0, :])
            proj_piece(0, 5, "act", ps=pp2[:, 1, :])
            finish_tail(heads.pop(0))
            for ec in range(EC):
                proj_piece(1, ec, "act")

    nc.compile()
    _CACHE["nc"] = nc
    return nc


def make_in_maps(x, w_attn, b_attn, w_proj, b_proj):
    x = np.ascontiguousarray(np.asarray(x, dtype=np.float32))
    w_attn = np.ascontiguousarray(np.asarray(w_attn, dtype=np.float32))
    b_attn = np.ascontiguousarray(np.asarray(b_attn, dtype=np.float32))
    w_proj = np.ascontiguousarray(np.asarray(w_proj, dtype=np.float32))
    b_proj = np.ascontiguousarray(np.asarray(b_proj, dtype=np.float32))

    bf = ml_dtypes.bfloat16
    wqk = np.ascontiguousarray(w_attn[:, : 2 * C].astype(bf))
    wv = np.ascontiguousarray(w_attn[:, 2 * C :].astype(bf))
    wpb = np.ascontiguousarray(w_proj.astype(bf))

    cstm = np.zeros((P, 18), dtype=np.float32)
    cstm[:, 0:JQK] = b_attn[: 2 * C].reshape(JQK, P).T
    cstm[:, JQK : JQK + EC] = b_proj.reshape(EC, P).T

    cstbm = np.zeros((P, 1024), dtype=np.float32)
    cstbm[:, 0:C] = np.tile(b_attn[2 * C :][None, :], (P, 1))
    # M01[a, b] = 1 where a < b (strict upper): mask matmul helper kept for
    # the PE-mask variant; negI = -4096 * I
    cstbm[:, C : C + P] = (
        np.arange(P)[:, None] < np.arange(P)[None, :]
    ).astype(np.float32)
    cstbm[:, C + P : C + 2 * P] = -4096.0 * np.eye(P, dtype=np.float32)

    csthm = np.zeros((2, 384), dtype=np.float32)
    csthm[0, 0:64] = 1.0      # sel2 row 0 -> prs rows 0:64
    csthm[1, 64:128] = 1.0    # sel2 row 1 -> prs rows 64:128
    csthm[0, 128:192] = 1.0   # selA -> prs rows 0:64
    csthm[0, 320:384] = 1.0   # selB -> prs rows 64:128

    shared = {
        "wqk": wqk,
        "wv": wv,
        "wp": wpb,
        "cst": cstm,
        "cstb": np.ascontiguousarray(cstbm.astype(bf)),
        "csth": np.ascontiguousarray(csthm.astype(np.float16)),
    }
    return [
        {"xT": np.ascontiguousarray(x[b].T.astype(bf)), **shared}
        for b in range(NCORES)
    ]


def kernel(**inputs):
    nc = _build()
    in_maps = make_in_maps(
        inputs["x"],
        inputs["w_attn"],
        inputs["b_attn"],
        inputs["w_proj"],
        inputs["b_proj"],
    )
    res = run_bass_kernel_spmd(nc, in_maps, list(range(NCORES)))
    out = np.stack(
        [
            np.ascontiguousarray(
                np.asarray(res.results[b]["yT"]).astype(np.float32).T
            )
            for b in range(NCORES)
        ]
    )
    return out.astype(np.float32)


# revision 30
# speedup vs baseline: 1.0376x; 1.0376x over previous
"""Causal self-attention (B=8, T=1024, C=768, NH=12) on 8 TRN2 NeuronCores.

Strategy: pure batch data-parallel - core b computes batch element b end to
end (no collectives).

v4 (baseline 266909 -> v2 248375 -> v3 165219 -> this):
  * all weights preloaded up front (12 wqk chunks + wv + wp) on the sync
    queue - no mid-phase weight DMA waits.
  * startup: x chunk 0 DMA'd first, qkt chunks 0/6 accumulate in x-arrival
    order (cc-pair interleaved across 4 psum groups), few small dummies
    warm the PE clock.
  * causal masks (gpsimd affine_select) are the ONLY gpsimd work;
    the softmax-denominator DMA hops moved to the sync queue (idle in the
    attention phase) so masks never queue behind them - this was the v3
    engine stall that tripped HAM down to half clock.
  * one spread matmul per block: recips land as a [2, TQ] fp16 tile
    (par0 row 0, par1 row 1), spread to [128, TQ] by a K=2 selector
    matmul; rawT even/odd halves are plain DVE muls (mixed partition
    offsets are legal) - v3's odd-half DMA reshape hop is gone.
  * last two blocks skip the DMA hops: direct 1-partition DVE recips
    (partition 64 -> 0) halve the tail chain latency.
  * block order: all (g, t2=0) halves first, then all (g, t2=1); proj
    t2=0 chunks become PE filler work late in the t2=1 phase, and two
    reserved proj chunks cover the final denominator chains.
  * evac engine split: qkt phase-A evacs + per-block par0 evac on ACT,
    par1 evac + norm muls + filler qkt evacs on DVE.
  * yT output in bf16 (host upcasts) - halves the output drain.

Per-core dataflow (everything kept "transposed", i.e. [feature, time]):
  xT [C, T] bf16                             (host pre-transposes x[b])
  qkT[j, t] = Wqk[:, j].T x                  attT-friendly layout
  v  [t, j] = x Wv                           AV-friendly layout, one ones
                                             column per head (denominator)
  attT[tk, tq] = kT.T @ qT   per head pair   PSUM [128, 2, 512]
  expT = exp(scale * attT)                   no max-sub: |logits| small
  diag blocks: affine_select (gpsimd)        causal mask
  out_aug[d|denom] = [v | 1].T @ expT        M=65, psum row 64 = denom
  rawT[j, t] = out_aug[d] * (1/denom)        K=2 selector spread matmul
  yT[e, t] = Wp.T @ rawT + bp                bf16 out, host transposes
"""

import os
import sys
from collections import deque

import numpy as np

for _p in ("/opt/trn_rl_repo", "/root/.axon_site/_ro/trn_rl_repo"):
    if os.path.isdir(_p) and _p not in sys.path:
        sys.path.insert(0, _p)

import ml_dtypes

import concourse.bacc as bacc
import concourse.mybir as mybir
import concourse.tile as tile
from concourse.bass import ts
from concourse.bass_utils import run_bass_kernel_spmd

B, T, C = 8, 1024, 768
NH, HD = 12, 64
P = 128
NCORES = 8
CC = 6                 # contraction chunks over C
JQK = 12               # output chunks for q|k
EC = 6                 # output chunks for the projection
TQ = 512               # moving-dim tile (max psum bank width)
NTQ = 2
NTK = 8                # key chunks
G = 6                  # head pairs (two 64-wide heads per 128 partitions)
VW = 2 * HD + 2        # 130: per-pair v layout [d_even(64), 1, d_odd(64), 1]
JV = 384               # v output tile width (3 head pairs)
SCALE = 1.0 / float(np.sqrt(HD))
NDUM = 6               # warmup dummy matmuls
DUMN = 512
POPS = 3               # filler pops per AV pair
F32 = mybir.dt.float32
BF16 = mybir.dt.bfloat16
F16 = mybir.dt.float16
AF = mybir.ActivationFunctionType
ADD = mybir.AluOpType.add
MULT = mybir.AluOpType.mult

_CACHE = {}


def _build():
    if "nc" in _CACHE:
        return _CACHE["nc"]

    nc = bacc.Bacc("TRN2", target_bir_lowering=False, debug=False)

    xT = nc.dram_tensor("xT", [C, T], BF16, kind="ExternalInput")
    wqk = nc.dram_tensor("wqk", [C, 2 * C], BF16, kind="ExternalInput")
    wv = nc.dram_tensor("wv", [C, C], BF16, kind="ExternalInput")
    wp = nc.dram_tensor("wp", [C, C], BF16, kind="ExternalInput")
    cst = nc.dram_tensor("cst", [P, 18], F32, kind="ExternalInput")
    cstb = nc.dram_tensor("cstb", [P, 1024], BF16, kind="ExternalInput")
    csth = nc.dram_tensor("csth", [2, 384], F16, kind="ExternalInput")
    yT = nc.dram_tensor("yT", [C, T], BF16, kind="ExternalOutput")

    xT_r = xT[:].rearrange("(h p) t -> p h t", p=P)  # h = 6 chunks
    wqk_r = wqk[:].rearrange("(o p) j -> p o j", p=P)
    wv_r = wv[:].rearrange("(o p) j -> p o j", p=P)
    wp_r = wp[:].rearrange("(o p) e -> p o e", p=P)
    yT_r = yT[:].rearrange("(o p) t -> p o t", p=P)

    with tile.TileContext(nc) as tc:
        with (
            tc.tile_pool(name="const", bufs=1) as constp,
            tc.tile_pool(name="xt", bufs=3) as xtp,
            tc.tile_pool(name="wqk", bufs=12) as wqkp,
            tc.tile_pool(name="wv", bufs=1) as wvp,
            tc.tile_pool(name="wp", bufs=1) as wpp,
            tc.tile_pool(name="qkt", bufs=1) as qkTp,
            tc.tile_pool(name="vaug", bufs=1) as vap,
            tc.tile_pool(name="raw", bufs=1) as rawp,
            tc.tile_pool(name="exp", bufs=5) as expp,
            tc.tile_pool(name="asb", bufs=3) as asbp,
            tc.tile_pool(name="rr", bufs=4) as rrp,
            tc.tile_pool(name="yt", bufs=3) as ytp,
            tc.tile_pool(name="psA", bufs=2, space="PSUM") as psA,
            tc.tile_pool(name="psB", bufs=4, space="PSUM") as psB,
        ):
            # ---- warmup: keep the PE clock gate open until real work ----
            dumw = constp.tile([1, DUMN], BF16)
            nc.gpsimd.memset(dumw[:], 0.0)
            dups = psB.tile([1, DUMN], F32, tag="mm", name="dups")
            for _ in range(NDUM):
                nc.tensor.matmul(
                    dups[0:1, :], dumw[0:1, 0:1], dumw[0:1, :],
                    start=True, stop=True,
                )

            # ---- all input DMAs, spread across 4 engine DMA rings -------
            # a single ring sustains well under the per-core HBM rate; the
            # startup-critical x chunks + first weights go wide in parallel
            xt2s = [
                xtp.tile([P, 2, T], BF16, tag="xt", name=f"xt{h}")
                for h in range(3)
            ]
            xts = [xt2s[cc // 2][:, cc % 2, :] for cc in range(CC)]

            wt = [
                wqkp.tile([P, CC, P], BF16, tag="wqk", name=f"wt{j}")
                for j in range(JQK)
            ]

            def load_wt(jc, eng=nc.sync):
                eng.dma_start(wt[jc][:], wqk_r[:, :, ts(jc, P)])

            load_wt(0, nc.scalar)
            nc.sync.dma_start(xt2s[0][:], xT_r[:, 0:2, :])
            load_wt(G, nc.scalar)
            nc.sync.dma_start(xt2s[1][:], xT_r[:, 2:4, :])
            nc.sync.dma_start(xt2s[2][:], xT_r[:, 4:6, :])

            wv_sb = wvp.tile([P, CC, C], BF16)
            nc.sync.dma_start(wv_sb[:, 0:3, :], wv_r[:, 0:3, :])
            nc.sync.dma_start(wv_sb[:, 3:6, :], wv_r[:, 3:6, :])
            cst_sb = constp.tile([P, 18], F32)
            nc.scalar.dma_start(cst_sb[:], cst[:])
            load_wt(1, nc.scalar)
            load_wt(G + 1, nc.scalar)
            # prime the ACT activation table during the DMA-paced startup
            # (the lazy ACT_TABLE_LOAD costs 1.3us on the first ACT op)
            dumact = constp.tile([1, 1], F32)
            nc.scalar.copy(dumact[:], dumw[0:1, 0:1])

            cstb_sb = constp.tile([P, 1024], BF16)
            nc.sync.dma_start(cstb_sb[:], cstb[:])
            csth_sb = constp.tile([2, 384], F16)
            nc.sync.dma_start(csth_sb[:], csth[:])
            bqk_sb = cst_sb[:, 0:JQK]
            bp_sb = cst_sb[:, JQK : JQK + EC]
            bv_sb = cstb_sb[:, 0:C]
            sel2 = csth_sb[0:2, 0:128]
            selA = csth_sb[0:1, 128:256]
            selB = csth_sb[0:1, 256:384]

            wp_sb = wpp.tile([P, CC, C], BF16)
            nc.sync.dma_start(wp_sb[:, 0:3, :], wp_r[:, 0:3, :])
            nc.sync.dma_start(wp_sb[:, 3:6, :], wp_r[:, 3:6, :])
            for j in (2, 8, 3, 9, 4, 10, 5, 11):
                load_wt(j, nc.sync)

            qkT_sb = qkTp.tile([P, JQK, T], BF16)
            v_sb = vap.tile([P, NTK, G * VW], BF16)
            v4 = v_sb[:].rearrange("p n (g w) -> p n g w", w=VW)
            rawT = rawp.tile([P, CC, T], BF16)

            # ones columns feed the softmax-denominator trick
            onec = constp.tile([P, 1], F32)
            nc.vector.memset(onec[:], 1.0)
            ones_src = onec[:, None, None, :].to_broadcast([P, NTK, G, 1])
            nc.vector.tensor_copy(v4[:, :, :, HD : HD + 1], ones_src)
            nc.vector.tensor_copy(v4[:, :, :, VW - 1 : VW], ones_src)

            # ---- qkt chunks 0/6 in x-arrival order ----------------------
            psq4 = {}
            for j in (0, G):
                for t2 in range(NTQ):
                    psq4[(j, t2)] = psB.tile(
                        [P, TQ], F32, tag="mm", name="psq"
                    )
            # emission order matches DMA arrival (wt0/wt6 land first)
            for j, p3 in ((0, 0), (G, 0), (0, 1), (G, 1), (0, 2), (G, 2)):
                for t2 in range(NTQ):
                    for cc in (2 * p3, 2 * p3 + 1):
                        nc.tensor.matmul(
                            psq4[(j, t2)][:],
                            wt[j][:, cc, :],
                            xts[cc][:, ts(t2, TQ)],
                            start=(cc == 0),
                            stop=(cc == CC - 1),
                        )
            for j in (0, G):
                for t2 in range(NTQ):
                    nc.scalar.add(
                        qkT_sb[:, j, ts(t2, TQ)],
                        psq4[(j, t2)][:],
                        bqk_sb[:, j : j + 1],
                    )

            # ---- v phase -----------------------------------------------
            for tc_i in range(NTK):
                for jn in range(C // JV):
                    ps = psB.tile([P, TQ], F32, tag="mm", name="psv")
                    for cc in range(CC):
                        nc.tensor.matmul(
                            ps[:, :JV],
                            xts[cc][:, ts(tc_i, P)],
                            wv_sb[:, cc, ts(jn, JV)],
                            start=(cc == 0),
                            stop=(cc == CC - 1),
                        )
                    g0 = jn * (JV // P)  # 3 head pairs per 384 cols
                    srcv = ps[:, :JV].rearrange(
                        "p (g h d) -> p g h d", h=2, d=HD
                    )
                    bias = bv_sb[:, ts(jn, JV)].rearrange(
                        "p (g h d) -> p g h d", h=2, d=HD
                    )
                    nc.vector.tensor_tensor(
                        v4[:, tc_i, g0 : g0 + 3, 0:HD],
                        srcv[:, :, 0, :],
                        bias[:, :, 0, :],
                        ADD,
                    )
                    nc.vector.tensor_tensor(
                        v4[:, tc_i, g0 : g0 + 3, HD + 1 : VW - 1],
                        srcv[:, :, 1, :],
                        bias[:, :, 1, :],
                        ADD,
                    )

            # ---- qkt chunks 1/7 ----------------------------------------
            def qkt_chunk(jc):
                for t2 in range(NTQ):
                    ps = psB.tile([P, TQ], F32, tag="mm", name="psq")
                    for cc in range(CC):
                        nc.tensor.matmul(
                            ps[:],
                            wt[jc][:, cc, :],
                            xts[cc][:, ts(t2, TQ)],
                            start=(cc == 0),
                            stop=(cc == CC - 1),
                        )
                    nc.scalar.add(
                        qkT_sb[:, jc, ts(t2, TQ)],
                        ps[:],
                        bqk_sb[:, jc : jc + 1],
                    )

            # ---- fillers: remaining qkt chunks + (later) proj t2=0 ------
            fillq = []

            remaining = {}

            def qkt_fillers(jc, t2):
                state = {}
                key = (jc, t2)
                remaining[key] = CC

                def mk(cc, state=state):
                    def run():
                        if cc == 0:
                            state["ps"] = psB.tile(
                                [P, TQ], F32, tag="mm", name="psqf"
                            )
                        ps = state["ps"]
                        nc.tensor.matmul(
                            ps[:],
                            wt[jc][:, cc, :],
                            xts[cc][:, ts(t2, TQ)],
                            start=(cc == 0),
                            stop=(cc == CC - 1),
                        )
                        if cc == CC - 1:
                            nc.vector.tensor_scalar_add(
                                qkT_sb[:, jc, ts(t2, TQ)],
                                ps[:],
                                bqk_sb[:, jc : jc + 1],
                            )

                    return run

                fillq.extend((key, mk(cc)) for cc in range(CC))

            for t2 in range(NTQ):  # all t2=0 halves first
                for g2 in (1, 2, 3, 4, 5):
                    qkt_fillers(g2, t2)
                    qkt_fillers(G + g2, t2)

            def pop_fill(k):
                for _ in range(k):
                    if fillq:
                        key, fn = fillq.pop(0)
                        if key is not None:
                            remaining[key] -= 1
                        fn()

            def drain_for(keys):
                # emit every filler a block depends on before its first QK
                while fillq and any(remaining.get(k, 0) > 0 for k in keys):
                    pop_fill(1)

            fill0 = nc.gpsimd.to_reg(0.0)

            def finish_head(state):
                """Denominator chain, no PE work: [P, 8] transpose hop on
                the sync queue, parallel fp16 recip, hop back to [2, TQ]."""
                g, t2, asb = state
                rd = rrp.tile([P, 8], F32, tag="rd", name="rd")
                nc.sync.dma_start(rd[:], asb[64:65, :, :])
                rd2 = rrp.tile([P, 8], F16, tag="rd2", name="rd2")
                with nc.allow_low_precision(
                    reason="fp16 softmax denominators keep 11 bits"
                ):
                    nc.vector.reciprocal(rd2[:], rd[:])
                rro2 = rrp.tile([2, TQ], F16, tag="rro2", name="rro2")
                nc.sync.dma_start(rro2[:], rd2[:])
                return g, t2, asb, rro2

            def finish_tail(h):
                """K=2 selector spread matmul + the two normalize muls."""
                g, t2, asb, rro2 = h
                prs = psB.tile([P, TQ], F32, tag="mm", name="prs")
                nc.tensor.matmul(prs[:], sel2, rro2[:], start=True, stop=True)
                nc.vector.tensor_mul(
                    rawT[0:64, g, ts(t2, TQ)], asb[0:64, 0, :], prs[0:64, :]
                )
                nc.vector.tensor_mul(
                    rawT[64:128, g, ts(t2, TQ)], asb[0:64, 1, :],
                    prs[64:128, :],
                )

            def attn_finish(state):
                finish_tail(finish_head(state))

            def proj_piece(t2, ec, eng, ps=None):
                if ps is None:
                    ps = psB.tile([P, TQ], F32, tag="mm", name="psp")
                for jc in range(CC):
                    nc.tensor.matmul(
                        ps[:],
                        wp_sb[:, jc, ts(ec, P)],
                        rawT[:, jc, ts(t2, TQ)],
                        start=(jc == 0),
                        stop=(jc == CC - 1),
                    )
                yt = ytp.tile([P, TQ], BF16, tag="yt", name="yt")
                if eng == "act":
                    nc.scalar.add(yt[:], ps[:], bp_sb[:, ec : ec + 1])
                else:
                    nc.vector.tensor_scalar_add(
                        yt[:], ps[:], bp_sb[:, ec : ec + 1]
                    )
                nc.sync.dma_start(yT_r[:, ec, ts(t2, TQ)], yt[:])

            def proj_fillers(t2, ec):
                state = {}

                def mk(jc, state=state):
                    def run():
                        if jc == 0:
                            state["ps"] = psB.tile(
                                [P, TQ], F32, tag="mm", name="pspf"
                            )
                        ps = state["ps"]
                        nc.tensor.matmul(
                            ps[:],
                            wp_sb[:, jc, ts(ec, P)],
                            rawT[:, jc, ts(t2, TQ)],
                            start=(jc == 0),
                            stop=(jc == CC - 1),
                        )
                        if jc == CC - 1:
                            yt = ytp.tile([P, TQ], BF16, tag="yt", name="yt")
                            nc.vector.tensor_scalar_add(
                                yt[:], ps[:], bp_sb[:, ec : ec + 1]
                            )
                            nc.sync.dma_start(
                                yT_r[:, ec, ts(t2, TQ)], yt[:]
                            )

                    return run

                fillq.extend((None, mk(jc)) for jc in range(CC))

            # ---- attention: flat pipeline across all 12 blocks ----------
            # The last AV pair of block n is emitted after block n+1's
            # first QK, so the PE never faces a block-start exp+mask chain
            # with nothing in front of it. Filler pops go BEFORE each
            # carried AV (in-order queue: work behind a waiting AV is
            # stuck, work in front is not).
            sched = [(g, 0) for g in range(G)] + [(g, 1) for g in range(G)]
            carryq = deque()   # (g, t2, hi, e, cs, tkc), AV lags 2 tkcs
            avs_cur = [None, None]
            heads = []         # denominator chains in flight
            ntail = 0

            def do_av(c):
                g, t2, hi, e, cs, tkc = c
                if tkc == 0:
                    avs_cur[0] = psB.tile([P, TQ], F32, tag="mm", name="av0")
                    avs_cur[1] = psB.tile([P, TQ], F32, tag="mm", name="av1")
                for par in (0, 1):
                    vlo = g * VW + (HD + 1) * par
                    nc.tensor.matmul(
                        avs_cur[par][0:65, cs:],
                        v_sb[:, tkc, vlo : vlo + HD + 1],
                        e[:, par, cs:],
                        start=(tkc == 0),
                        stop=(tkc == hi - 1),
                    )
                if tkc == hi - 1:
                    asb = asbp.tile(
                        [65, 2, TQ], F32, tag="avsb", name="asb"
                    )
                    nc.vector.tensor_scalar_add(
                        asb[:, 0, :], avs_cur[0][0:65, :], 0.0
                    )
                    nc.vector.tensor_scalar_add(
                        asb[:, 1, :], avs_cur[1][0:65, :], 0.0
                    )
                    # start the denominator chain immediately (no PE work)
                    heads.append(finish_head((g, t2, asb)))

            for bi, (g, t2) in enumerate(sched):
                jq, jk = g, G + g
                hi = 4 * (t2 + 1)  # causal: key chunks 0..hi-1
                need = [(g, t2), (G + g, t2)]
                if t2 == 1:
                    need.append((G + g, 0))
                drain_for(need)
                for tkc in range(hi):
                    csr = tkc * P - t2 * TQ  # diag block start col
                    cs = max(0, csr)
                    pa = psA.tile([P, 2, TQ], F32, tag="pa", name="pa")
                    for par in (0, 1):
                        qrow = HD * par
                        nc.tensor.matmul(
                            pa[:, par, cs:TQ],
                            qkT_sb[qrow : qrow + HD, jk, ts(tkc, P)],
                            qkT_sb[
                                qrow : qrow + HD,
                                jq,
                                t2 * TQ + cs : (t2 + 1) * TQ,
                            ],
                            start=True,
                            stop=True,
                        )
                    pop_fill(2 if tkc < 2 else 1)
                    e = expp.tile([P, 2, TQ], BF16, tag="exp", name="e")
                    nc.scalar.activation(
                        e[:, :, cs:], pa[:, :, cs:], AF.Exp, scale=SCALE
                    )
                    if csr >= 0:
                        # causal mask on the diagonal 128-wide block: keep
                        # e[tk, tq] only where tq_local >= tk_local
                        nc.gpsimd.affine_select(
                            e[:, :, cs : cs + P],
                            e[:, :, cs : cs + P],
                            pattern=[[0, 2], [1, P]],
                            compare_op=mybir.AluOpType.is_ge,
                            fill=fill0,
                            base=0,
                            channel_multiplier=-1,
                        )
                    if len(carryq) == 3:
                        do_av(carryq.popleft())
                        if tkc < 3:
                            pop_fill(1)
                    carryq.append((g, t2, hi, e, cs, tkc))
                    if len(heads) >= 2 and ntail < 10:
                        finish_tail(heads.pop(0))
                        ntail += 1
                if bi == 9:
                    for ec in range(2):
                        proj_fillers(0, ec)
            while carryq:
                do_av(carryq.popleft())
            # tail: both remaining chains are already in flight; keep the
            # PE busy on reserved proj chunks while they run. Those chunks
            # accumulate in retired pa (psA) banks so they never wait on
            # the psB ring still entangled with the last block's AV banks.
            pop_fill(999)
            pp1 = psA.tile([P, 2, TQ], F32, tag="pa", name="pp1")
            pp2 = psA.tile([P, 2, TQ], F32, tag="pa", name="pp2")
            proj_piece(0, 2, "act", ps=pp1[:, 0, :])
            proj_piece(0, 3, "act", ps=pp1[:, 1, :])
            finish_tail(heads.pop(0))
            proj_piece(0, 4, "act", ps=pp2[:, 0, :])
            proj_piece(0, 5, "act", ps=pp2[:, 1, :])
            finish_tail(heads.pop(0))
            for ec in range(EC):
                proj_piece(1, ec, "act")

    nc.compile()
    _CACHE["nc"] = nc
    return nc


def make_in_maps(x, w_attn, b_attn, w_proj, b_proj):
    x = np.ascontiguousarray(np.asarray(x, dtype=np.float32))
    w_attn = np.ascontiguousarray(np.asarray(w_attn, dtype=np.float32))
    b_attn = np.ascontiguousarray(np.asarray(b_attn, dtype=np.float32))
    w_proj = np.ascontiguousarray(np.asarray(w_proj, dtype=np.float32))
    b_proj = np.ascontiguousarray(np.asarray(b_proj, dtype=np.float32))

    bf = ml_dtypes.bfloat16
    wqk = np.ascontiguousarray(w_attn[:, : 2 * C].astype(bf))
    wv = np.ascontiguousarray(w_attn[:, 2 * C :].astype(bf))
    wpb = np.ascontiguousarray(w_proj.astype(bf))

    cstm = np.zeros((P, 18), dtype=np.float32)
    cstm[:, 0:JQK] = b_attn[: 2 * C].reshape(JQK, P).T
    cstm[:, JQK : JQK + EC] = b_proj.reshape(EC, P).T

    cstbm = np.zeros((P, 1024), dtype=np.float32)
    cstbm[:, 0:C] = np.tile(b_attn[2 * C :][None, :], (P, 1))
    # M01[a, b] = 1 where a < b (strict upper): mask matmul helper kept for
    # the PE-mask variant; negI = -4096 * I
    cstbm[:, C : C + P] = (
        np.arange(P)[:, None] < np.arange(P)[None, :]
    ).astype(np.float32)
    cstbm[:, C + P : C + 2 * P] = -4096.0 * np.eye(P, dtype=np.float32)

    csthm = np.zeros((2, 384), dtype=np.float32)
    csthm[0, 0:64] = 1.0      # sel2 row 0 -> prs rows 0:64
    csthm[1, 64:128] = 1.0    # sel2 row 1 -> prs rows 64:128
    csthm[0, 128:192] = 1.0   # selA -> prs rows 0:64
    csthm[0, 320:384] = 1.0   # selB -> prs rows 64:128

    shared = {
        "wqk": wqk,
        "wv": wv,
        "wp": wpb,
        "cst": cstm,
        "cstb": np.ascontiguousarray(cstbm.astype(bf)),
        "csth": np.ascontiguousarray(csthm.astype(np.float16)),
    }
    return [
        {"xT": np.ascontiguousarray(x[b].T.astype(bf)), **shared}
        for b in range(NCORES)
    ]


def kernel(**inputs):
    nc = _build()
    in_maps = make_in_maps(
        inputs["x"],
        inputs["w_attn"],
        inputs["b_attn"],
        inputs["w_proj"],
        inputs["b_proj"],
    )
    res = run_bass_kernel_spmd(nc, in_maps, list(range(NCORES)))
    out = np.stack(
        [
            np.ascontiguousarray(
                np.asarray(res.results[b]["yT"]).astype(np.float32).T
            )
            for b in range(NCORES)
        ]
    )
    return out.astype(np.float32)


# revision 31
# speedup vs baseline: 1.0383x; 1.0006x over previous
"""Causal self-attention (B=8, T=1024, C=768, NH=12) on 8 TRN2 NeuronCores.

Strategy: pure batch data-parallel - core b computes batch element b end to
end (no collectives).

v4 (baseline 266909 -> v2 248375 -> v3 165219 -> this):
  * all weights preloaded up front (12 wqk chunks + wv + wp) on the sync
    queue - no mid-phase weight DMA waits.
  * startup: x chunk 0 DMA'd first, qkt chunks 0/6 accumulate in x-arrival
    order (cc-pair interleaved across 4 psum groups), few small dummies
    warm the PE clock.
  * causal masks (gpsimd affine_select) are the ONLY gpsimd work;
    the softmax-denominator DMA hops moved to the sync queue (idle in the
    attention phase) so masks never queue behind them - this was the v3
    engine stall that tripped HAM down to half clock.
  * one spread matmul per block: recips land as a [2, TQ] fp16 tile
    (par0 row 0, par1 row 1), spread to [128, TQ] by a K=2 selector
    matmul; rawT even/odd halves are plain DVE muls (mixed partition
    offsets are legal) - v3's odd-half DMA reshape hop is gone.
  * last two blocks skip the DMA hops: direct 1-partition DVE recips
    (partition 64 -> 0) halve the tail chain latency.
  * block order: all (g, t2=0) halves first, then all (g, t2=1); proj
    t2=0 chunks become PE filler work late in the t2=1 phase, and two
    reserved proj chunks cover the final denominator chains.
  * evac engine split: qkt phase-A evacs + per-block par0 evac on ACT,
    par1 evac + norm muls + filler qkt evacs on DVE.
  * yT output in bf16 (host upcasts) - halves the output drain.

Per-core dataflow (everything kept "transposed", i.e. [feature, time]):
  xT [C, T] bf16                             (host pre-transposes x[b])
  qkT[j, t] = Wqk[:, j].T x                  attT-friendly layout
  v  [t, j] = x Wv                           AV-friendly layout, one ones
                                             column per head (denominator)
  attT[tk, tq] = kT.T @ qT   per head pair   PSUM [128, 2, 512]
  expT = exp(scale * attT)                   no max-sub: |logits| small
  diag blocks: affine_select (gpsimd)        causal mask
  out_aug[d|denom] = [v | 1].T @ expT        M=65, psum row 64 = denom
  rawT[j, t] = out_aug[d] * (1/denom)        K=2 selector spread matmul
  yT[e, t] = Wp.T @ rawT + bp                bf16 out, host transposes
"""

import os
import sys
from collections import deque

import numpy as np

for _p in ("/opt/trn_rl_repo", "/root/.axon_site/_ro/trn_rl_repo"):
    if os.path.isdir(_p) and _p not in sys.path:
        sys.path.insert(0, _p)

import ml_dtypes

import concourse.bacc as bacc
import concourse.mybir as mybir
import concourse.tile as tile
from concourse.bass import ts
from concourse.bass_utils import run_bass_kernel_spmd

B, T, C = 8, 1024, 768
NH, HD = 12, 64
P = 128
NCORES = 8
CC = 6                 # contraction chunks over C
JQK = 12               # output chunks for q|k
EC = 6                 # output chunks for the projection
TQ = 512               # moving-dim tile (max psum bank width)
NTQ = 2
NTK = 8                # key chunks
G = 6                  # head pairs (two 64-wide heads per 128 partitions)
VW = 2 * HD + 2        # 130: per-pair v layout [d_even(64), 1, d_odd(64), 1]
JV = 384               # v output tile width (3 head pairs)
SCALE = 1.0 / float(np.sqrt(HD))
NDUM = 6               # warmup dummy matmuls
DUMN = 512
POPS = 3               # filler pops per AV pair
F32 = mybir.dt.float32
BF16 = mybir.dt.bfloat16
F16 = mybir.dt.float16
AF = mybir.ActivationFunctionType
ADD = mybir.AluOpType.add
MULT = mybir.AluOpType.mult

_CACHE = {}


def _build():
    if "nc" in _CACHE:
        return _CACHE["nc"]

    nc = bacc.Bacc("TRN2", target_bir_lowering=False, debug=False)

    xT = nc.dram_tensor("xT", [C, T], BF16, kind="ExternalInput")
    wqk = nc.dram_tensor("wqk", [C, 2 * C], BF16, kind="ExternalInput")
    wv = nc.dram_tensor("wv", [C, C], BF16, kind="ExternalInput")
    wp = nc.dram_tensor("wp", [C, C], BF16, kind="ExternalInput")
    cst = nc.dram_tensor("cst", [P, 18], F32, kind="ExternalInput")
    cstb = nc.dram_tensor("cstb", [P, 1024], BF16, kind="ExternalInput")
    csth = nc.dram_tensor("csth", [2, 384], F16, kind="ExternalInput")
    yT = nc.dram_tensor("yT", [C, T], BF16, kind="ExternalOutput")

    xT_r = xT[:].rearrange("(h p) t -> p h t", p=P)  # h = 6 chunks
    wqk_r = wqk[:].rearrange("(o p) j -> p o j", p=P)
    wv_r = wv[:].rearrange("(o p) j -> p o j", p=P)
    wp_r = wp[:].rearrange("(o p) e -> p o e", p=P)
    yT_r = yT[:].rearrange("(o p) t -> p o t", p=P)

    with tile.TileContext(nc) as tc:
        with (
            tc.tile_pool(name="const", bufs=1) as constp,
            tc.tile_pool(name="xt", bufs=3) as xtp,
            tc.tile_pool(name="wqk", bufs=12) as wqkp,
            tc.tile_pool(name="wv", bufs=1) as wvp,
            tc.tile_pool(name="wp", bufs=1) as wpp,
            tc.tile_pool(name="qkt", bufs=1) as qkTp,
            tc.tile_pool(name="vaug", bufs=1) as vap,
            tc.tile_pool(name="raw", bufs=1) as rawp,
            tc.tile_pool(name="exp", bufs=5) as expp,
            tc.tile_pool(name="asb", bufs=3) as asbp,
            tc.tile_pool(name="rr", bufs=4) as rrp,
            tc.tile_pool(name="yt", bufs=3) as ytp,
            tc.tile_pool(name="psA", bufs=2, space="PSUM") as psA,
            tc.tile_pool(name="psB", bufs=4, space="PSUM") as psB,
        ):
            # ---- warmup: keep the PE clock gate open until real work ----
            dumw = constp.tile([1, DUMN], BF16)
            nc.gpsimd.memset(dumw[:], 0.0)
            dups = psB.tile([1, DUMN], F32, tag="mm", name="dups")
            for _ in range(NDUM):
                nc.tensor.matmul(
                    dups[0:1, :], dumw[0:1, 0:1], dumw[0:1, :],
                    start=True, stop=True,
                )

            # ---- all input DMAs, spread across 4 engine DMA rings -------
            # a single ring sustains well under the per-core HBM rate; the
            # startup-critical x chunks + first weights go wide in parallel
            xt2s = [
                xtp.tile([P, 2, T], BF16, tag="xt", name=f"xt{h}")
                for h in range(3)
            ]
            xts = [xt2s[cc // 2][:, cc % 2, :] for cc in range(CC)]

            wt = [
                wqkp.tile([P, CC, P], BF16, tag="wqk", name=f"wt{j}")
                for j in range(JQK)
            ]

            def load_wt(jc, eng=nc.sync):
                eng.dma_start(wt[jc][:], wqk_r[:, :, ts(jc, P)])

            load_wt(0, nc.scalar)
            nc.sync.dma_start(xts[0], xT_r[:, 0, :])
            nc.sync.dma_start(xts[1], xT_r[:, 1, :])
            load_wt(G, nc.scalar)
            nc.sync.dma_start(xts[2], xT_r[:, 2, :])
            nc.sync.dma_start(xts[3], xT_r[:, 3, :])
            nc.sync.dma_start(xts[4], xT_r[:, 4, :])
            nc.sync.dma_start(xts[5], xT_r[:, 5, :])

            wv_sb = wvp.tile([P, CC, C], BF16)
            nc.sync.dma_start(wv_sb[:, 0:3, :], wv_r[:, 0:3, :])
            nc.sync.dma_start(wv_sb[:, 3:6, :], wv_r[:, 3:6, :])
            cst_sb = constp.tile([P, 18], F32)
            nc.scalar.dma_start(cst_sb[:], cst[:])
            load_wt(1, nc.scalar)
            load_wt(G + 1, nc.scalar)
            # prime the ACT activation table during the DMA-paced startup
            # (the lazy ACT_TABLE_LOAD costs 1.3us on the first ACT op)
            dumact = constp.tile([1, 1], F32)
            nc.scalar.copy(dumact[:], dumw[0:1, 0:1])

            cstb_sb = constp.tile([P, 1024], BF16)
            nc.sync.dma_start(cstb_sb[:], cstb[:])
            csth_sb = constp.tile([2, 384], F16)
            nc.sync.dma_start(csth_sb[:], csth[:])
            bqk_sb = cst_sb[:, 0:JQK]
            bp_sb = cst_sb[:, JQK : JQK + EC]
            bv_sb = cstb_sb[:, 0:C]
            sel2 = csth_sb[0:2, 0:128]
            selA = csth_sb[0:1, 128:256]
            selB = csth_sb[0:1, 256:384]

            wp_sb = wpp.tile([P, CC, C], BF16)
            nc.sync.dma_start(wp_sb[:, 0:3, :], wp_r[:, 0:3, :])
            nc.sync.dma_start(wp_sb[:, 3:6, :], wp_r[:, 3:6, :])
            for j in (2, 8, 3, 9, 4, 10, 5, 11):
                load_wt(j, nc.sync)

            qkT_sb = qkTp.tile([P, JQK, T], BF16)
            v_sb = vap.tile([P, NTK, G * VW], BF16)
            v4 = v_sb[:].rearrange("p n (g w) -> p n g w", w=VW)
            rawT = rawp.tile([P, CC, T], BF16)

            # ones columns feed the softmax-denominator trick
            onec = constp.tile([P, 1], F32)
            nc.vector.memset(onec[:], 1.0)
            ones_src = onec[:, None, None, :].to_broadcast([P, NTK, G, 1])
            nc.vector.tensor_copy(v4[:, :, :, HD : HD + 1], ones_src)
            nc.vector.tensor_copy(v4[:, :, :, VW - 1 : VW], ones_src)

            # ---- qkt chunks 0/6 in x-arrival order ----------------------
            psq4 = {}
            for j in (0, G):
                for t2 in range(NTQ):
                    psq4[(j, t2)] = psB.tile(
                        [P, TQ], F32, tag="mm", name="psq"
                    )
            # emission order matches DMA arrival (wt0/wt6 land first)
            for j, p3 in ((0, 0), (G, 0), (0, 1), (G, 1), (0, 2), (G, 2)):
                for t2 in range(NTQ):
                    for cc in (2 * p3, 2 * p3 + 1):
                        nc.tensor.matmul(
                            psq4[(j, t2)][:],
                            wt[j][:, cc, :],
                            xts[cc][:, ts(t2, TQ)],
                            start=(cc == 0),
                            stop=(cc == CC - 1),
                        )
            for j in (0, G):
                for t2 in range(NTQ):
                    nc.scalar.add(
                        qkT_sb[:, j, ts(t2, TQ)],
                        psq4[(j, t2)][:],
                        bqk_sb[:, j : j + 1],
                    )

            # ---- v phase -----------------------------------------------
            for tc_i in range(NTK):
                for jn in range(C // JV):
                    ps = psB.tile([P, TQ], F32, tag="mm", name="psv")
                    for cc in range(CC):
                        nc.tensor.matmul(
                            ps[:, :JV],
                            xts[cc][:, ts(tc_i, P)],
                            wv_sb[:, cc, ts(jn, JV)],
                            start=(cc == 0),
                            stop=(cc == CC - 1),
                        )
                    g0 = jn * (JV // P)  # 3 head pairs per 384 cols
                    srcv = ps[:, :JV].rearrange(
                        "p (g h d) -> p g h d", h=2, d=HD
                    )
                    bias = bv_sb[:, ts(jn, JV)].rearrange(
                        "p (g h d) -> p g h d", h=2, d=HD
                    )
                    nc.vector.tensor_tensor(
                        v4[:, tc_i, g0 : g0 + 3, 0:HD],
                        srcv[:, :, 0, :],
                        bias[:, :, 0, :],
                        ADD,
                    )
                    nc.vector.tensor_tensor(
                        v4[:, tc_i, g0 : g0 + 3, HD + 1 : VW - 1],
                        srcv[:, :, 1, :],
                        bias[:, :, 1, :],
                        ADD,
                    )

            # ---- qkt chunks 1/7 ----------------------------------------
            def qkt_chunk(jc):
                for t2 in range(NTQ):
                    ps = psB.tile([P, TQ], F32, tag="mm", name="psq")
                    for cc in range(CC):
                        nc.tensor.matmul(
                            ps[:],
                            wt[jc][:, cc, :],
                            xts[cc][:, ts(t2, TQ)],
                            start=(cc == 0),
                            stop=(cc == CC - 1),
                        )
                    nc.scalar.add(
                        qkT_sb[:, jc, ts(t2, TQ)],
                        ps[:],
                        bqk_sb[:, jc : jc + 1],
                    )

            # ---- fillers: remaining qkt chunks + (later) proj t2=0 ------
            fillq = []

            remaining = {}

            def qkt_fillers(jc, t2):
                state = {}
                key = (jc, t2)
                remaining[key] = CC

                def mk(cc, state=state):
                    def run():
                        if cc == 0:
                            state["ps"] = psB.tile(
                                [P, TQ], F32, tag="mm", name="psqf"
                            )
                        ps = state["ps"]
                        nc.tensor.matmul(
                            ps[:],
                            wt[jc][:, cc, :],
                            xts[cc][:, ts(t2, TQ)],
                            start=(cc == 0),
                            stop=(cc == CC - 1),
                        )
                        if cc == CC - 1:
                            nc.vector.tensor_scalar_add(
                                qkT_sb[:, jc, ts(t2, TQ)],
                                ps[:],
                                bqk_sb[:, jc : jc + 1],
                            )

                    return run

                fillq.extend((key, mk(cc)) for cc in range(CC))

            for t2 in range(NTQ):  # all t2=0 halves first
                for g2 in (1, 2, 3, 4, 5):
                    qkt_fillers(g2, t2)
                    qkt_fillers(G + g2, t2)

            def pop_fill(k):
                for _ in range(k):
                    if fillq:
                        key, fn = fillq.pop(0)
                        if key is not None:
                            remaining[key] -= 1
                        fn()

            def drain_for(keys):
                # emit every filler a block depends on before its first QK
                while fillq and any(remaining.get(k, 0) > 0 for k in keys):
                    pop_fill(1)

            fill0 = nc.gpsimd.to_reg(0.0)

            def finish_head(state):
                """Denominator chain, no PE work: [P, 8] transpose hop on
                the sync queue, parallel fp16 recip, hop back to [2, TQ]."""
                g, t2, asb = state
                rd = rrp.tile([P, 8], F32, tag="rd", name="rd")
                nc.sync.dma_start(rd[:], asb[64:65, :, :])
                rd2 = rrp.tile([P, 8], F16, tag="rd2", name="rd2")
                with nc.allow_low_precision(
                    reason="fp16 softmax denominators keep 11 bits"
                ):
                    nc.vector.reciprocal(rd2[:], rd[:])
                rro2 = rrp.tile([2, TQ], F16, tag="rro2", name="rro2")
                nc.sync.dma_start(rro2[:], rd2[:])
                return g, t2, asb, rro2

            def finish_tail(h):
                """K=2 selector spread matmul + the two normalize muls."""
                g, t2, asb, rro2 = h
                prs = psB.tile([P, TQ], F32, tag="mm", name="prs")
                nc.tensor.matmul(prs[:], sel2, rro2[:], start=True, stop=True)
                nc.vector.tensor_mul(
                    rawT[0:64, g, ts(t2, TQ)], asb[0:64, 0, :], prs[0:64, :]
                )
                nc.vector.tensor_mul(
                    rawT[64:128, g, ts(t2, TQ)], asb[0:64, 1, :],
                    prs[64:128, :],
                )

            def attn_finish(state):
                finish_tail(finish_head(state))

            def proj_piece(t2, ec, eng, ps=None):
                if ps is None:
                    ps = psB.tile([P, TQ], F32, tag="mm", name="psp")
                for jc in range(CC):
                    nc.tensor.matmul(
                        ps[:],
                        wp_sb[:, jc, ts(ec, P)],
                        rawT[:, jc, ts(t2, TQ)],
                        start=(jc == 0),
                        stop=(jc == CC - 1),
                    )
                yt = ytp.tile([P, TQ], BF16, tag="yt", name="yt")
                if eng == "act":
                    nc.scalar.add(yt[:], ps[:], bp_sb[:, ec : ec + 1])
                else:
                    nc.vector.tensor_scalar_add(
                        yt[:], ps[:], bp_sb[:, ec : ec + 1]
                    )
                nc.sync.dma_start(yT_r[:, ec, ts(t2, TQ)], yt[:])

            def proj_fillers(t2, ec):
                state = {}

                def mk(jc, state=state):
                    def run():
                        if jc == 0:
                            state["ps"] = psB.tile(
                                [P, TQ], F32, tag="mm", name="pspf"
                            )
                        ps = state["ps"]
                        nc.tensor.matmul(
                            ps[:],
                            wp_sb[:, jc, ts(ec, P)],
                            rawT[:, jc, ts(t2, TQ)],
                            start=(jc == 0),
                            stop=(jc == CC - 1),
                        )
                        if jc == CC - 1:
                            yt = ytp.tile([P, TQ], BF16, tag="yt", name="yt")
                            nc.vector.tensor_scalar_add(
                                yt[:], ps[:], bp_sb[:, ec : ec + 1]
                            )
                            nc.sync.dma_start(
                                yT_r[:, ec, ts(t2, TQ)], yt[:]
                            )

                    return run

                fillq.extend((None, mk(jc)) for jc in range(CC))

            # ---- attention: flat pipeline across all 12 blocks ----------
            # The last AV pair of block n is emitted after block n+1's
            # first QK, so the PE never faces a block-start exp+mask chain
            # with nothing in front of it. Filler pops go BEFORE each
            # carried AV (in-order queue: work behind a waiting AV is
            # stuck, work in front is not).
            sched = [(g, 0) for g in range(G)] + [(g, 1) for g in range(G)]
            carryq = deque()   # (g, t2, hi, e, cs, tkc), AV lags 2 tkcs
            avs_cur = [None, None]
            heads = []         # denominator chains in flight
            ntail = 0

            def do_av(c):
                g, t2, hi, e, cs, tkc = c
                if tkc == 0:
                    avs_cur[0] = psB.tile([P, TQ], F32, tag="mm", name="av0")
                    avs_cur[1] = psB.tile([P, TQ], F32, tag="mm", name="av1")
                for par in (0, 1):
                    vlo = g * VW + (HD + 1) * par
                    nc.tensor.matmul(
                        avs_cur[par][0:65, cs:],
                        v_sb[:, tkc, vlo : vlo + HD + 1],
                        e[:, par, cs:],
                        start=(tkc == 0),
                        stop=(tkc == hi - 1),
                    )
                if tkc == hi - 1:
                    asb = asbp.tile(
                        [65, 2, TQ], F32, tag="avsb", name="asb"
                    )
                    nc.vector.tensor_scalar_add(
                        asb[:, 0, :], avs_cur[0][0:65, :], 0.0
                    )
                    nc.vector.tensor_scalar_add(
                        asb[:, 1, :], avs_cur[1][0:65, :], 0.0
                    )
                    # start the denominator chain immediately (no PE work)
                    heads.append(finish_head((g, t2, asb)))

            for bi, (g, t2) in enumerate(sched):
                jq, jk = g, G + g
                hi = 4 * (t2 + 1)  # causal: key chunks 0..hi-1
                need = [(g, t2), (G + g, t2)]
                if t2 == 1:
                    need.append((G + g, 0))
                drain_for(need)
                for tkc in range(hi):
                    csr = tkc * P - t2 * TQ  # diag block start col
                    cs = max(0, csr)
                    pa = psA.tile([P, 2, TQ], F32, tag="pa", name="pa")
                    for par in (0, 1):
                        qrow = HD * par
                        nc.tensor.matmul(
                            pa[:, par, cs:TQ],
                            qkT_sb[qrow : qrow + HD, jk, ts(tkc, P)],
                            qkT_sb[
                                qrow : qrow + HD,
                                jq,
                                t2 * TQ + cs : (t2 + 1) * TQ,
                            ],
                            start=True,
                            stop=True,
                        )
                    pop_fill(2 if tkc < 2 else 1)
                    e = expp.tile([P, 2, TQ], BF16, tag="exp", name="e")
                    nc.scalar.activation(
                        e[:, :, cs:], pa[:, :, cs:], AF.Exp, scale=SCALE
                    )
                    if csr >= 0:
                        # causal mask on the diagonal 128-wide block: keep
                        # e[tk, tq] only where tq_local >= tk_local
                        nc.gpsimd.affine_select(
                            e[:, :, cs : cs + P],
                            e[:, :, cs : cs + P],
                            pattern=[[0, 2], [1, P]],
                            compare_op=mybir.AluOpType.is_ge,
                            fill=fill0,
                            base=0,
                            channel_multiplier=-1,
                        )
                    if len(carryq) == 3:
                        do_av(carryq.popleft())
                        if tkc < 3:
                            pop_fill(1)
                    carryq.append((g, t2, hi, e, cs, tkc))
                    if len(heads) >= 2 and ntail < 10:
                        finish_tail(heads.pop(0))
                        ntail += 1
                if bi == 9:
                    for ec in range(2):
                        proj_fillers(0, ec)
            while carryq:
                do_av(carryq.popleft())
            # tail: both remaining chains are already in flight; keep the
            # PE busy on reserved proj chunks while they run. Those chunks
            # accumulate in retired pa (psA) banks so they never wait on
            # the psB ring still entangled with the last block's AV banks.
            pop_fill(999)
            pp1 = psA.tile([P, 2, TQ], F32, tag="pa", name="pp1")
            pp2 = psA.tile([P, 2, TQ], F32, tag="pa", name="pp2")
            proj_piece(0, 2, "act", ps=pp1[:, 0, :])
            proj_piece(0, 3, "act", ps=pp1[:, 1, :])
            finish_tail(heads.pop(0))
            proj_piece(0, 4, "act", ps=pp2[:, 0, :])
            proj_piece(0, 5, "act", ps=pp2[:, 1, :])
            finish_tail(heads.pop(0))
            for ec in range(EC):
                proj_piece(1, ec, "act")

    nc.compile()
    _CACHE["nc"] = nc
    return nc


def make_in_maps(x, w_attn, b_attn, w_proj, b_proj):
    x = np.ascontiguousarray(np.asarray(x, dtype=np.float32))
    w_attn = np.ascontiguousarray(np.asarray(w_attn, dtype=np.float32))
    b_attn = np.ascontiguousarray(np.asarray(b_attn, dtype=np.float32))
    w_proj = np.ascontiguousarray(np.asarray(w_proj, dtype=np.float32))
    b_proj = np.ascontiguousarray(np.asarray(b_proj, dtype=np.float32))

    bf = ml_dtypes.bfloat16
    wqk = np.ascontiguousarray(w_attn[:, : 2 * C].astype(bf))
    wv = np.ascontiguousarray(w_attn[:, 2 * C :].astype(bf))
    wpb = np.ascontiguousarray(w_proj.astype(bf))

    cstm = np.zeros((P, 18), dtype=np.float32)
    cstm[:, 0:JQK] = b_attn[: 2 * C].reshape(JQK, P).T
    cstm[:, JQK : JQK + EC] = b_proj.reshape(EC, P).T

    cstbm = np.zeros((P, 1024), dtype=np.float32)
    cstbm[:, 0:C] = np.tile(b_attn[2 * C :][None, :], (P, 1))
    # M01[a, b] = 1 where a < b (strict upper): mask matmul helper kept for
    # the PE-mask variant; negI = -4096 * I
    cstbm[:, C : C + P] = (
        np.arange(P)[:, None] < np.arange(P)[None, :]
    ).astype(np.float32)
    cstbm[:, C + P : C + 2 * P] = -4096.0 * np.eye(P, dtype=np.float32)

    csthm = np.zeros((2, 384), dtype=np.float32)
    csthm[0, 0:64] = 1.0      # sel2 row 0 -> prs rows 0:64
    csthm[1, 64:128] = 1.0    # sel2 row 1 -> prs rows 64:128
    csthm[0, 128:192] = 1.0   # selA -> prs rows 0:64
    csthm[0, 320:384] = 1.0   # selB -> prs rows 64:128

    shared = {
        "wqk": wqk,
        "wv": wv,
        "wp": wpb,
        "cst": cstm,
        "cstb": np.ascontiguousarray(cstbm.astype(bf)),
        "csth": np.ascontiguousarray(csthm.astype(np.float16)),
    }
    return [
        {"xT": np.ascontiguousarray(x[b].T.astype(bf)), **shared}
        for b in range(NCORES)
    ]


def kernel(**inputs):
    nc = _build()
    in_maps = make_in_maps(
        inputs["x"],
        inputs["w_attn"],
        inputs["b_attn"],
        inputs["w_proj"],
        inputs["b_proj"],
    )
    res = run_bass_kernel_spmd(nc, in_maps, list(range(NCORES)))
    out = np.stack(
        [
            np.ascontiguousarray(
                np.asarray(res.results[b]["yT"]).astype(np.float32).T
            )
            for b in range(NCORES)
        ]
    )
    return out.astype(np.float32)
